# revision 5
# baseline (speedup 1.0000x reference)
"""Trainium2 kernel for nn_HadamardRotation: y = x @ H, H = 4096x4096 Walsh-Hadamard.

Strategy
--------
H4096 = H64 (x) H64 (Kronecker). Writing d = 64*hi + lo, e = 64*hi' + lo':

    y[r, e] = sum_{hi,lo} H64[lo,lo'] * H64[hi,hi'] * x[r, d]

Two matmul stages with 128-wide contraction (block-diagonal I2 (x) H64
weights), separated by an on-chip "corner turn" (SBUF->SBUF DMA partition
shuffle), all operating in the transposed domain (d on partitions, rows on
the free axis). Host does the cheap transposes / index unscrambles; the
device only ever issues contiguous >=1KB DMA lines.

FLOPs: 2 * 128/4096 of the naive matmul = 16x reduction.

Data parallel over 8 cores: rows sharded 16384 -> 8 x 2048, weights
replicated.

Layouts (per core, R = 2048 rows):
  xt  DRAM in  (32, 128, R): xt[a, 64*mu+lo, r] = x[r, 128*a + 64*mu + lo]
  B1  (128,128): B1[64*mu+lo, 2*lo'+mu]     = H64[lo, lo']
  B2  (128,128): B2[64*nu+32*mu+a, 2*hi'+nu] = H64[2*a+mu, hi']
  stage A (chunk a): u_a[p, r] = sum_k B1[k, p] xt[a, k, r]
      => u_a[4c + (2*nu+mu)] holds (hi = 2a+mu, lo' = 2c+nu)
  corner turn:  v_c[32*t + a, r] = u_a[4*c + t, r]
  stage B (chunk c): Y[c, m, r] = sum_q B2[q, m] v_c[q, r]
      => Y[c, 2*hi'+nu, r] = y[r, 64*hi' + 2*c + nu]
"""

import math
import numpy as np
import ml_dtypes

import concourse.bass as bass
import concourse.mybir as mybir
import concourse.tile as tile
from concourse import bacc
from concourse.bass_utils import run_bass_kernel_spmd

N_CORES = 8
DIM = 4096
R_TOTAL = 4 * 4096          # rows after flattening (4, 4096, DIM)
R = R_TOTAL // N_CORES      # rows per core
N = 512                     # free-dim slab (one PSUM bank of fp32)
SLABS = R // N

# dtype mode: "fp32" (exact, PE 4 cyc/row), "fp32r" (fp32 storage, fast PE
# mode), "bf16" (half storage+DMA for x/intermediate, exact weights)
MODE = "bf16"

# tuning knobs (overridable for benching)
CFG = dict(
    ycopy="vector2",   # engine for psum->sbuf copy of stage-B out: vector|any|vector2 (split DVE/ACT)
    ucopy="vector",    # engine for psum->sbuf copy of stage-A out
    turn_eng="scalar",  # corner-turn DMA engine: scalar|sync|gpsimd|rr (round robin)
    in_eng="sync",
    out_eng="sync",
    in_batch=4,        # chunks per input DMA
    out_batch=4,       # batch output DMAs over this many c-chunks
    turn_slabs=1,      # how many N-slabs share one corner-turn DMA
    pipeline=1,        # emit stage A of slab s+1 before stage B of slab s
    ycast=0,           # stage-B out staged as bf16 in SBUF, SWDGE casts to f32
    out_bf16=1,        # Y stored bf16 in HBM; host upcasts to f32
    xbufs=3, ubufs=2, vbufs=4, ybufs=4,
)


def _walsh_hadamard64():
    h = np.array([[1.0]], dtype=np.float64)
    while h.shape[0] < 64:
        h = np.block([[h, h], [h, -h]]) / math.sqrt(2.0)
    return h.astype(np.float32)


def _build_weights(H64):
    B1 = np.zeros((128, 128), dtype=np.float32)
    b1v = B1.reshape(2, 64, 64, 2)
    for mu in range(2):
        b1v[mu, :, :, mu] = H64
    B2 = np.zeros((128, 128), dtype=np.float32)
    b2v = B2.reshape(2, 2, 32, 64, 2)
    for nu in range(2):
        for mu in range(2):
            b2v[nu, mu, :, :, nu] = H64[mu::2, :]
    return B1, B2


_NC_CACHE = {}


def _build_bass(mode, loop=0, cfg=None):
    cfg = dict(CFG, **(cfg or {}))
    key = (mode, loop, tuple(sorted(cfg.items())))
    if key in _NC_CACHE:
        return _NC_CACHE[key]

    f32 = mybir.dt.float32
    dt_in = mybir.dt.bfloat16 if mode == "bf16" else f32
    mm_cast = (lambda ap: ap.bitcast(mybir.dt.float32r)) if mode == "fp32r" else (lambda ap: ap)

    dt_out = mybir.dt.bfloat16 if cfg.get("out_bf16") else f32

    nc = bacc.Bacc("TRN2", target_bir_lowering=False, debug=False,
                   num_devices=N_CORES)
    xt_d = nc.dram_tensor("xt", [32, 128, R], dt_in, kind="ExternalInput")
    B1_d = nc.dram_tensor("B1", [128, 128], dt_in, kind="ExternalInput")
    B2_d = nc.dram_tensor("B2", [128, 128], dt_in, kind="ExternalInput")
    Y_d = nc.dram_tensor("Y", [32, 128, R], dt_out, kind="ExternalOutput")

    OB = cfg["out_batch"]

    with tile.TileContext(nc) as tc:
        with (
            tc.tile_pool(name="wpool", bufs=1) as wpool,
            tc.tile_pool(name="xpool", bufs=cfg["xbufs"]) as xpool,
            tc.tile_pool(name="upool", bufs=cfg["ubufs"]) as upool,
            tc.tile_pool(name="vpool", bufs=cfg["vbufs"]) as vpool,
            tc.tile_pool(name="ypool", bufs=cfg["ybufs"]) as ypool,
            tc.tile_pool(name="psA", bufs=4, space="PSUM") as psA,
            tc.tile_pool(name="psB", bufs=4, space="PSUM") as psB,
        ):
            B1_sb = wpool.tile([128, 128], dt_in)
            nc.sync.dma_start(B1_sb[:], B1_d[:])
            B2_sb = wpool.tile([128, 128], dt_in)
            nc.sync.dma_start(B2_sb[:], B2_d[:])

            in_eng = getattr(nc, cfg["in_eng"])
            out_eng = getattr(nc, cfg["out_eng"])
            turn_eng = None if cfg["turn_eng"] == "rr" else getattr(nc, cfg["turn_eng"])

            def copy(engine, dst, src, i):
                if engine == "vector":
                    nc.vector.tensor_copy(dst, src)
                elif engine == "vector2":
                    # alternate DVE / ACT so neither engine binds
                    if i % 2 == 0:
                        nc.vector.tensor_copy(dst, src)
                    else:
                        nc.any.tensor_copy(dst, src)
                else:
                    nc.any.tensor_copy(dst, src)

            turn_rr = [nc.scalar, nc.sync, nc.gpsimd]

            def turn(i):
                if cfg["turn_eng"] == "rr":
                    return turn_rr[i % 3]
                return turn_eng

            TS = cfg["turn_slabs"]
            IB = cfg["in_batch"]

            def phaseA(sg):
                    u_all = upool.tile([128, 32, TS * N], dt_in)
                    for ts in range(TS):
                        s = sg * TS + ts
                        ns = slice(s * N, (s + 1) * N)
                        for g in range(32 // IB):
                            xg = xpool.tile([128, IB, N], dt_in)
                            in_eng.dma_start(
                                xg[:],
                                xt_d[IB * g:IB * (g + 1), :, ns].transpose([1, 0, 2]))
                            for j in range(IB):
                                a = IB * g + j
                                pu = psA.tile([128, N], f32)
                                nc.tensor.matmul(pu[:], mm_cast(B1_sb[:]),
                                                 mm_cast(xg[:, j, :]),
                                                 start=True, stop=True)
                                copy(cfg["ucopy"],
                                     u_all[:, a, ts * N:(ts + 1) * N], pu[:], a)
                    return u_all

            def phaseB(sg, u_all):
                    # corner turn + stage B
                    ut = u_all.tensor
                    PU = u_all.ap[0][0]  # partition stride in elements
                    L = TS * N
                    if cfg.get("out_bf16"):
                        dt_y = mybir.dt.bfloat16
                        y_eng = out_eng
                    else:
                        dt_y = mybir.dt.bfloat16 if cfg["ycast"] else f32
                        y_eng = nc.gpsimd if cfg["ycast"] else out_eng
                    for cb in range(32 // OB):
                        ybs = [ypool.tile([128, OB, N], dt_y, name=f"yb{ts}")
                               for ts in range(TS)]
                        for j in range(OB):
                            c = cb * OB + j
                            vc = vpool.tile([128, L], dt_in)
                            in_ap = bass.AP(ut, 4 * c * PU,
                                            [[PU, 4], [L, 32], [1, L]])
                            turn(c).dma_start(vc[:], in_ap)
                            for ts in range(TS):
                                py = psB.tile([128, N], f32)
                                nc.tensor.matmul(py[:], mm_cast(B2_sb[:]),
                                                 mm_cast(vc[:, ts * N:(ts + 1) * N]),
                                                 start=True, stop=True)
                                copy(cfg["ycopy"], ybs[ts][:, j, :], py[:], c + ts)
                        for ts in range(TS):
                            s = sg * TS + ts
                            y_eng.dma_start(
                                Y_d[cb * OB:(cb + 1) * OB, :,
                                    s * N:(s + 1) * N].transpose([1, 0, 2]),
                                ybs[ts][:])

            def body():
                if cfg["pipeline"]:
                    # software pipeline: emit stage A of slab-group sg+1
                    # before stage B of sg, so PE never stalls on the turn.
                    pending = None
                    for sg in range(SLABS // TS):
                        u_all = phaseA(sg)
                        if pending is not None:
                            phaseB(*pending)
                        pending = (sg, u_all)
                    phaseB(*pending)
                else:
                    for sg in range(SLABS // TS):
                        phaseB(sg, phaseA(sg))

            if loop:
                with tc.For_i(0, loop, 1):
                    body()
            else:
                body()

    nc.compile()
    _NC_CACHE[key] = nc
    return nc


def _prep_inputs(x, H, mode):
    np_in = ml_dtypes.bfloat16 if mode == "bf16" else np.float32
    H64 = (np.asarray(H, dtype=np.float32)[::64, ::64] * 8.0).astype(np.float32)
    B1, B2 = _build_weights(H64)
    B1 = B1.astype(np_in)
    B2 = B2.astype(np_in)
    xf = np.asarray(x, dtype=np.float32).reshape(R_TOTAL, DIM)
    in_maps = []
    for i in range(N_CORES):
        shard = xf[i * R:(i + 1) * R]                     # (R, DIM)
        xt = np.ascontiguousarray(shard.T, dtype=np_in)   # (DIM, R)
        xt = xt.reshape(32, 128, R)
        in_maps.append({"xt": xt, "B1": B1, "B2": B2})
    return in_maps


def _unscramble(results):
    outs = []
    for i in range(N_CORES):
        Y = results[i]["Y"]                               # (32, 128, R) f32
        y = Y.reshape(32, 64, 2, R).transpose(3, 1, 0, 2).reshape(R, DIM)
        outs.append(y)
    return np.concatenate(outs, axis=0).reshape(4, 4096, DIM).astype(np.float32)


def kernel(x, H, _trace=False, _loop=0, _cfg=None):
    nc = _build_bass(MODE, loop=_loop, cfg=_cfg)
    in_maps = _prep_inputs(x, H, MODE)
    res = run_bass_kernel_spmd(nc, in_maps, core_ids=list(range(N_CORES)),
                               trace=_trace)
    out = _unscramble(res.results)
    if _trace:
        return out, res
    return out



# revision 10
# speedup vs baseline: 1.0513x; 1.0513x over previous
"""Trainium2 kernel for nn_HadamardRotation: y = x @ H, H = 4096x4096 Walsh-Hadamard.

Strategy
--------
H4096 = H64 (x) H64 (Kronecker). Writing d = 64*hi + lo, e = 64*hi' + lo':

    y[r, e] = sum_{hi,lo} H64[lo,lo'] * H64[hi,hi'] * x[r, d]

Two matmul stages with 128-wide contraction (block-diagonal I2 (x) H64
weights), separated by an on-chip "corner turn" (SBUF->SBUF DMA partition
shuffle), all operating in the transposed domain (d on partitions, rows on
the free axis). Host does the cheap transposes / index unscrambles; the
device only ever issues contiguous >=1KB DMA lines.

FLOPs: 2 * 128/4096 of the naive matmul = 16x reduction.

Data parallel over 8 cores: rows sharded 16384 -> 8 x 2048, weights
replicated.

Layouts (per core, R = 2048 rows):
  xt  DRAM in  (32, 128, R): xt[a, 64*mu+lo, r] = x[r, 128*a + 64*mu + lo]
  B1  (128,128): B1[64*mu+lo, 2*lo'+mu]     = H64[lo, lo']
  B2  (128,128): B2[64*nu+32*mu+a, 2*hi'+nu] = H64[2*a+mu, hi']
  stage A (chunk a): u_a[p, r] = sum_k B1[k, p] xt[a, k, r]
      => u_a[4c + (2*nu+mu)] holds (hi = 2a+mu, lo' = 2c+nu)
  corner turn:  v_c[32*t + a, r] = u_a[4*c + t, r]
  stage B (chunk c): Y[c, m, r] = sum_q B2[q, m] v_c[q, r]
      => Y[c, 2*hi'+nu, r] = y[r, 64*hi' + 2*c + nu]
"""

import math
import numpy as np
import ml_dtypes

import concourse.bass as bass
import concourse.mybir as mybir
import concourse.tile as tile
from concourse import bacc
from concourse.bass_utils import run_bass_kernel_spmd

N_CORES = 8
DIM = 4096
R_TOTAL = 4 * 4096          # rows after flattening (4, 4096, DIM)
R = R_TOTAL // N_CORES      # rows per core
N = 512                     # free-dim slab (one PSUM bank of fp32)
SLABS = R // N

# dtype mode: "fp32" (exact, PE 4 cyc/row), "fp32r" (fp32 storage, fast PE
# mode), "bf16" (half storage+DMA for x/intermediate, exact weights)
MODE = "bf16"

# tuning knobs (overridable for benching)
CFG = dict(
    ycopy="vector2",   # engine for psum->sbuf copy of stage-B out: vector|any|vector2 (split DVE/ACT)
    ucopy="vector",    # engine for psum->sbuf copy of stage-A out
    turn_eng="scalar",  # corner-turn DMA engine: scalar|sync|gpsimd|rr (round robin)
    in_eng="sync",
    out_eng="sync",
    in_batch=4,        # chunks per input DMA
    out_batch=4,       # batch output DMAs over this many c-chunks
    turn_slabs=1,      # how many N-slabs share one corner-turn DMA
    pipeline=1,        # emit stage A of slab s+1 before stage B of slab s
    ycast=0,           # stage-B out staged as bf16 in SBUF, SWDGE casts to f32
    out_bf16=1,        # Y stored bf16 in HBM; host upcasts to f32
    xbufs=3, ubufs=2, vbufs=4, ybufs=4,
    # ablation knobs (break correctness; for HW component timing only)
    skip_in=0, skip_a=0, skip_turn=0, skip_b=0, skip_out=0,
)


def _walsh_hadamard64():
    h = np.array([[1.0]], dtype=np.float64)
    while h.shape[0] < 64:
        h = np.block([[h, h], [h, -h]]) / math.sqrt(2.0)
    return h.astype(np.float32)


def _build_weights(H64):
    B1 = np.zeros((128, 128), dtype=np.float32)
    b1v = B1.reshape(2, 64, 64, 2)
    for mu in range(2):
        b1v[mu, :, :, mu] = H64
    B2 = np.zeros((128, 128), dtype=np.float32)
    b2v = B2.reshape(2, 2, 32, 64, 2)
    for nu in range(2):
        for mu in range(2):
            b2v[nu, mu, :, :, nu] = H64[mu::2, :]
    return B1, B2


_NC_CACHE = {}


def _build_bass(mode, loop=0, cfg=None):
    cfg = dict(CFG, **(cfg or {}))
    key = (mode, loop, tuple(sorted(cfg.items())))
    if key in _NC_CACHE:
        return _NC_CACHE[key]

    f32 = mybir.dt.float32
    dt_in = mybir.dt.bfloat16 if mode == "bf16" else f32
    mm_cast = (lambda ap: ap.bitcast(mybir.dt.float32r)) if mode == "fp32r" else (lambda ap: ap)

    dt_out = mybir.dt.bfloat16 if cfg.get("out_bf16") else f32

    nc = bacc.Bacc("TRN2", target_bir_lowering=False, debug=False,
                   num_devices=N_CORES)
    xt_d = nc.dram_tensor("xt", [32, 128, R], dt_in, kind="ExternalInput")
    B1_d = nc.dram_tensor("B1", [128, 128], dt_in, kind="ExternalInput")
    B2_d = nc.dram_tensor("B2", [128, 128], dt_in, kind="ExternalInput")
    Y_d = nc.dram_tensor("Y", [32, 128, R], dt_out, kind="ExternalOutput")

    OB = cfg["out_batch"]

    with tile.TileContext(nc) as tc:
        with (
            tc.tile_pool(name="wpool", bufs=1) as wpool,
            tc.tile_pool(name="xpool", bufs=cfg["xbufs"]) as xpool,
            tc.tile_pool(name="upool", bufs=cfg["ubufs"]) as upool,
            tc.tile_pool(name="vpool", bufs=cfg["vbufs"]) as vpool,
            tc.tile_pool(name="ypool", bufs=cfg["ybufs"]) as ypool,
            tc.tile_pool(name="psA", bufs=4, space="PSUM") as psA,
            tc.tile_pool(name="psB", bufs=4, space="PSUM") as psB,
        ):
            B1_sb = wpool.tile([128, 128], dt_in)
            nc.sync.dma_start(B1_sb[:], B1_d[:])
            B2_sb = wpool.tile([128, 128], dt_in)
            nc.sync.dma_start(B2_sb[:], B2_d[:])

            in_eng = getattr(nc, cfg["in_eng"])
            out_eng = getattr(nc, cfg["out_eng"])
            turn_eng = None if cfg["turn_eng"] == "rr" else getattr(nc, cfg["turn_eng"])

            def copy(engine, dst, src, i):
                if engine == "vector":
                    nc.vector.tensor_copy(dst, src)
                elif engine == "scalar":
                    nc.scalar.copy(dst, src)
                elif engine == "alt":
                    # alternate DVE / ACT explicitly so neither engine binds
                    if i % 2 == 0:
                        nc.vector.tensor_copy(dst, src)
                    else:
                        nc.scalar.copy(dst, src)
                elif engine == "vector2":
                    # alternate DVE / ACT so neither engine binds
                    if i % 2 == 0:
                        nc.vector.tensor_copy(dst, src)
                    else:
                        nc.any.tensor_copy(dst, src)
                else:
                    nc.any.tensor_copy(dst, src)

            turn_rr = [nc.scalar, nc.sync, nc.gpsimd]

            def turn(i):
                if cfg["turn_eng"] == "rr":
                    return turn_rr[i % 3]
                return turn_eng

            TS = cfg["turn_slabs"]
            IB = cfg["in_batch"]

            def phaseA(sg):
                    u_all = upool.tile([128, 32, TS * N], dt_in)
                    for ts in range(TS):
                        s = sg * TS + ts
                        ns = slice(s * N, (s + 1) * N)
                        for g in range(32 // IB):
                            xg = xpool.tile([128, IB, N], dt_in)
                            if not cfg["skip_in"]:
                                in_eng.dma_start(
                                    xg[:],
                                    xt_d[IB * g:IB * (g + 1), :, ns].transpose([1, 0, 2]))
                            if cfg["skip_a"]:
                                continue
                            for j in range(IB):
                                a = IB * g + j
                                pu = psA.tile([128, N], f32)
                                nc.tensor.matmul(pu[:], mm_cast(B1_sb[:]),
                                                 mm_cast(xg[:, j, :]),
                                                 start=True, stop=True)
                                copy(cfg["ucopy"],
                                     u_all[:, a, ts * N:(ts + 1) * N], pu[:], a)
                    return u_all

            def phaseB(sg, u_all):
                    # corner turn + stage B
                    ut = u_all.tensor
                    PU = u_all.ap[0][0]  # partition stride in elements
                    L = TS * N
                    if cfg.get("out_bf16"):
                        dt_y = mybir.dt.bfloat16
                        y_eng = out_eng
                    else:
                        dt_y = mybir.dt.bfloat16 if cfg["ycast"] else f32
                        y_eng = nc.gpsimd if cfg["ycast"] else out_eng
                    for cb in range(32 // OB):
                        ybs = [ypool.tile([128, OB, N], dt_y, name=f"yb{ts}")
                               for ts in range(TS)]
                        for j in range(OB):
                            c = cb * OB + j
                            vc = vpool.tile([128, L], dt_in)
                            if not cfg["skip_turn"]:
                                in_ap = bass.AP(ut, 4 * c * PU,
                                                [[PU, 4], [L, 32], [1, L]])
                                turn(c).dma_start(vc[:], in_ap)
                            if cfg["skip_b"]:
                                continue
                            for ts in range(TS):
                                py = psB.tile([128, N], f32)
                                nc.tensor.matmul(py[:], mm_cast(B2_sb[:]),
                                                 mm_cast(vc[:, ts * N:(ts + 1) * N]),
                                                 start=True, stop=True)
                                copy(cfg["ycopy"], ybs[ts][:, j, :], py[:], c + ts)
                        if cfg["skip_out"] or cfg["skip_b"]:
                            continue
                        for ts in range(TS):
                            s = sg * TS + ts
                            y_eng.dma_start(
                                Y_d[cb * OB:(cb + 1) * OB, :,
                                    s * N:(s + 1) * N].transpose([1, 0, 2]),
                                ybs[ts][:])

            def body():
                if cfg["pipeline"]:
                    # software pipeline: emit stage A of slab-group sg+1
                    # before stage B of sg, so PE never stalls on the turn.
                    pending = None
                    for sg in range(SLABS // TS):
                        u_all = phaseA(sg)
                        if pending is not None:
                            phaseB(*pending)
                        pending = (sg, u_all)
                    phaseB(*pending)
                else:
                    for sg in range(SLABS // TS):
                        phaseB(sg, phaseA(sg))

            if loop:
                with tc.For_i(0, loop, 1):
                    body()
            else:
                body()

    nc.compile()
    _NC_CACHE[key] = nc
    return nc


def _prep_inputs(x, H, mode):
    np_in = ml_dtypes.bfloat16 if mode == "bf16" else np.float32
    H64 = (np.asarray(H, dtype=np.float32)[::64, ::64] * 8.0).astype(np.float32)
    B1, B2 = _build_weights(H64)
    B1 = B1.astype(np_in)
    B2 = B2.astype(np_in)
    xf = np.asarray(x, dtype=np.float32).reshape(R_TOTAL, DIM)
    in_maps = []
    for i in range(N_CORES):
        shard = xf[i * R:(i + 1) * R]                     # (R, DIM)
        xt = np.ascontiguousarray(shard.T, dtype=np_in)   # (DIM, R)
        xt = xt.reshape(32, 128, R)
        in_maps.append({"xt": xt, "B1": B1, "B2": B2})
    return in_maps


def _unscramble(results):
    outs = []
    for i in range(N_CORES):
        Y = results[i]["Y"]                               # (32, 128, R) f32
        y = Y.reshape(32, 64, 2, R).transpose(3, 1, 0, 2).reshape(R, DIM)
        outs.append(y)
    return np.concatenate(outs, axis=0).reshape(4, 4096, DIM).astype(np.float32)


def kernel(x, H, _trace=False, _loop=0, _cfg=None):
    nc = _build_bass(MODE, loop=_loop, cfg=_cfg)
    in_maps = _prep_inputs(x, H, MODE)
    res = run_bass_kernel_spmd(nc, in_maps, core_ids=list(range(N_CORES)),
                               trace=_trace)
    out = _unscramble(res.results)
    if _trace:
        return out, res
    return out



# revision 22
# speedup vs baseline: 1.7987x; 1.7109x over previous
"""Trainium2 kernel for nn_HadamardRotation: y = x @ H, H = 4096x4096 Walsh-Hadamard.

Strategy
--------
H4096 = H64 (x) H64 (Kronecker). Writing d = 64*hi + lo, e = 64*hi' + lo':

    y[r, e] = sum_{hi,lo} H64[lo,lo'] * H64[hi,hi'] * x[r, d]

Two matmul stages with 128-wide contraction (block-diagonal I2 (x) H64
weights), separated by an on-chip "corner turn" (SBUF->SBUF DMA partition
shuffle), all operating in the transposed domain (d on partitions, rows on
the free axis). Host does the cheap transposes / index unscrambles; the
device only ever issues big contiguous DMAs:

  - input:  one 4 MB DMA per slab (contiguous per partition)
  - turn:   one DMA per slab per MC-group; the (t, a) partition-regroup
    collapses to a 3-dim AP because a-count * a-stride == partition pitch
  - output: one DMA per slab per OB-group (contiguous per partition)

FLOPs: 2 * 128/4096 of the naive matmul = 16x reduction.

Data parallel over 8 cores: rows sharded 16384 -> 8 x 2048, weights
replicated. x and Y are staged in HBM as bf16 (host casts).

Layouts (per core, R = 2048 rows, N = 512, SLABS = 4):
  xt  DRAM in  (SLABS, 128, 32*N): xt[s, 64*mu+lo, a*N+n] = x[s*N+n, 128*a+64*mu+lo]
  B1  (128,128): B1[64*mu+lo, 2*lo'+mu]      = H64[lo, lo']
  B2  (128,128): B2[64*nu+32*mu+a, 2*hi'+nu] = H64[2*a+mu, hi']
  stage A (chunk a): u[4c + (2*nu+mu), a*N+n] = sum_k B1[k, .] xt[s, k, a*N+n]
      => holds (hi = 2a+mu, lo' = 2c+nu)
  corner turn:  v[32*t + a, c, n] = u[4*c + t, a*N + n]
  stage B (chunk c): Y[2*hi'+nu, n] = sum_q B2[q, .] v[q, c, n]
      => y[s*N+n, 64*hi' + 2*c + nu]
  Y   DRAM out (SLABS, 32//OB, 128, OB*N):
      Y[s, cb, 2*hi'+nu, j*N+n] = y[s*N+n, 64*hi' + 2*(cb*OB+j) + nu]
"""

import math
import numpy as np
import ml_dtypes

import concourse.bass as bass
import concourse.mybir as mybir
import concourse.tile as tile
from concourse import bacc
from concourse.bass_utils import run_bass_kernel_spmd

N_CORES = 8
DIM = 4096
R_TOTAL = 4 * 4096          # rows after flattening (4, 4096, DIM)
R = R_TOTAL // N_CORES      # rows per core
N = 512                     # free-dim slab (one PSUM bank of fp32)
SLABS = R // N

MODE = "bf16"

CFG = dict(
    ycopy="alt",       # engine(s) for psum->sbuf copy of stage-B out
    ucopy="alt",       # engine(s) for psum->sbuf copy of stage-A out
    turn_eng="scalar",  # corner-turn DMA engine: scalar|sync|gpsimd|rr
    in_eng="sync",
    out_eng="sync",
    merge_c=16,        # c-chunks per corner-turn DMA (sb2sb path: must be 1)
    out_batch=16,      # c-chunks per output DMA
    pipeline=1,        # emit stage A of slab s+1 before stage B of slab s
    out_bf16=1,        # Y stored bf16 in HBM; host upcasts to f32
    turn_hbm=1,        # corner turn via HBM round-trip (big 3-dim DMAs)
    uout_eng="scalar",  # u -> HBM engine (same ring as vin => ordering)
    vin_eng="scalar",  # turned v <- HBM engine
    xbufs=2, ubufs=2, vbufs=2, ybufs=2,
    # ablation knobs (break correctness; for HW component timing only)
    skip_in=0, skip_a=0, skip_turn=0, skip_b=0, skip_out=0,
)


def _walsh_hadamard64():
    h = np.array([[1.0]], dtype=np.float64)
    while h.shape[0] < 64:
        h = np.block([[h, h], [h, -h]]) / math.sqrt(2.0)
    return h.astype(np.float32)


def _build_weights(H64):
    B1 = np.zeros((128, 128), dtype=np.float32)
    b1v = B1.reshape(2, 64, 64, 2)
    for mu in range(2):
        b1v[mu, :, :, mu] = H64
    B2 = np.zeros((128, 128), dtype=np.float32)
    b2v = B2.reshape(2, 2, 32, 64, 2)
    for nu in range(2):
        for mu in range(2):
            b2v[nu, mu, :, :, nu] = H64[mu::2, :]
    return B1, B2


_NC_CACHE = {}


def _build_bass(mode, loop=0, cfg=None):
    cfg = dict(CFG, **(cfg or {}))
    key = (mode, loop, tuple(sorted(cfg.items())))
    if key in _NC_CACHE:
        return _NC_CACHE[key]

    f32 = mybir.dt.float32
    dt_in = mybir.dt.bfloat16 if mode == "bf16" else f32
    mm_cast = (lambda ap: ap.bitcast(mybir.dt.float32r)) if mode == "fp32r" else (lambda ap: ap)

    OB = cfg["out_batch"]
    MC = cfg["merge_c"]
    NCB = 32 // OB
    dt_out = mybir.dt.bfloat16 if cfg.get("out_bf16") else f32

    nc = bacc.Bacc("TRN2", target_bir_lowering=False, debug=False,
                   num_devices=N_CORES)
    xt_d = nc.dram_tensor("xt", [SLABS, 128, 32 * N], dt_in, kind="ExternalInput")
    B1_d = nc.dram_tensor("B1", [128, 128], dt_in, kind="ExternalInput")
    B2_d = nc.dram_tensor("B2", [128, 128], dt_in, kind="ExternalInput")
    Y_d = nc.dram_tensor("Y", [SLABS, NCB, 128, OB * N], dt_out,
                         kind="ExternalOutput")
    ut_d = (nc.dram_tensor("uturn", [2, 128, 32 * N], dt_in, kind="Internal")
            if cfg["turn_hbm"] else None)

    with tile.TileContext(nc) as tc:
        with (
            tc.tile_pool(name="wpool", bufs=1) as wpool,
            tc.tile_pool(name="xpool", bufs=cfg["xbufs"]) as xpool,
            tc.tile_pool(name="upool", bufs=cfg["ubufs"]) as upool,
            tc.tile_pool(name="vpool", bufs=cfg["vbufs"]) as vpool,
            tc.tile_pool(name="ypool", bufs=cfg["ybufs"]) as ypool,
            tc.tile_pool(name="psA", bufs=4, space="PSUM") as psA,
            tc.tile_pool(name="psB", bufs=4, space="PSUM") as psB,
        ):
            B1_sb = wpool.tile([128, 128], dt_in)
            nc.sync.dma_start(B1_sb[:], B1_d[:])
            B2_sb = wpool.tile([128, 128], dt_in)
            nc.sync.dma_start(B2_sb[:], B2_d[:])

            in_eng = getattr(nc, cfg["in_eng"])
            out_eng = getattr(nc, cfg["out_eng"])
            turn_eng = None if cfg["turn_eng"] == "rr" else getattr(nc, cfg["turn_eng"])

            def copy(engine, dst, src, i):
                if engine == "vector":
                    nc.vector.tensor_copy(dst, src)
                elif engine == "scalar":
                    nc.scalar.copy(dst, src)
                elif engine == "alt":
                    if i % 2 == 0:
                        nc.vector.tensor_copy(dst, src)
                    else:
                        nc.scalar.copy(dst, src)
                else:
                    nc.any.tensor_copy(dst, src)

            turn_rr = [nc.scalar, nc.sync, nc.gpsimd]

            def turn(i):
                if cfg["turn_eng"] == "rr":
                    return turn_rr[i % 3]
                return turn_eng

            def phaseA(s):
                if cfg["skip_a"] and cfg["skip_turn"]:
                    u_all = None
                else:
                    u_all = upool.tile([128, 32, N], dt_in)
                    if cfg["skip_a"]:
                        nc.vector.memset(u_all[:, 0, 0:64], 0)
                if not (cfg["skip_in"] and cfg["skip_a"]):
                    xg = xpool.tile([128, 32, N], dt_in)
                    if not cfg["skip_in"]:
                        in_eng.dma_start(xg[:], xt_d[s])
                    if not cfg["skip_a"]:
                        for a in range(32):
                            pu = psA.tile([128, N], f32)
                            nc.tensor.matmul(pu[:], mm_cast(B1_sb[:]),
                                             mm_cast(xg[:, a, :]),
                                             start=True, stop=True)
                            copy(cfg["ucopy"], u_all[:, a, :], pu[:], a)
                return u_all

            def phaseB(s, u_all):
                if u_all is None:
                    return
                ut = u_all.tensor
                PU = u_all.ap[0][0]  # partition stride in elements
                dt_y = dt_out
                vgs = {}

                if cfg["turn_hbm"] and not cfg["skip_turn"]:
                    getattr(nc, cfg["uout_eng"]).dma_start(
                        ut_d[s % 2], u_all[:])

                def get_vc(c):
                    g = c // MC
                    if g not in vgs:
                        vg = vpool.tile([128, MC, N], dt_in)
                        if cfg["turn_hbm"]:
                            # turned read from the HBM staging copy of u:
                            # flat DRAM AP has no partition-step limit, so
                            # (t, a) merges into one 128-long stride-N dim.
                            utt = ut_d[:].tensor
                            base = (s % 2) * 128 * 32 * N + 4 * g * MC * 32 * N
                            in_ap = bass.AP(utt, base,
                                            [[N, 128], [4 * 32 * N, MC], [1, N]])
                            getattr(nc, cfg["vin_eng"]).dma_start(vg[:], in_ap)
                        else:
                            # sb2sb path: 2 partition dims + 1 free (MC must be 1)
                            in_ap = bass.AP(ut, 4 * g * MC * PU,
                                            [[PU, 4], [N, 32], [1, N]])
                            turn(g).dma_start(vg[:], in_ap)
                        vgs[g] = vg
                    return vgs[g][:, c % MC, :]

                for cb in range(NCB):
                    if cfg["skip_b"]:
                        if not cfg["skip_turn"]:
                            for j in range(OB):
                                get_vc(cb * OB + j)
                        continue
                    yb = ypool.tile([128, OB, N], dt_y)
                    for j in range(OB):
                        c = cb * OB + j
                        vc = get_vc(c) if not cfg["skip_turn"] else None
                        py = psB.tile([128, N], f32)
                        nc.tensor.matmul(py[:], mm_cast(B2_sb[:]),
                                         mm_cast(vc),
                                         start=True, stop=True)
                        copy(cfg["ycopy"], yb[:, j, :], py[:], c)
                    if not cfg["skip_out"]:
                        out_eng.dma_start(Y_d[s, cb], yb[:])

            def body():
                if cfg["pipeline"]:
                    pending = None
                    for s in range(SLABS):
                        u_all = phaseA(s)
                        if pending is not None:
                            phaseB(*pending)
                        pending = (s, u_all)
                    phaseB(*pending)
                else:
                    for s in range(SLABS):
                        phaseB(s, phaseA(s))

            if loop:
                with tc.For_i(0, loop, 1):
                    body()
            else:
                body()

    nc.compile()
    _NC_CACHE[key] = nc
    return nc


def _prep_inputs(x, H, mode, cfg=None):
    cfg = dict(CFG, **(cfg or {}))
    np_in = ml_dtypes.bfloat16 if mode == "bf16" else np.float32
    H64 = (np.asarray(H, dtype=np.float32)[::64, ::64] * 8.0).astype(np.float32)
    B1, B2 = _build_weights(H64)
    B1 = B1.astype(np_in)
    B2 = B2.astype(np_in)
    xf = np.asarray(x, dtype=np.float32).reshape(R_TOTAL, DIM)
    in_maps = []
    for i in range(N_CORES):
        shard = xf[i * R:(i + 1) * R]                     # (R, DIM)
        # xt[s, p, a*N+n] = shard[s*N+n, 128*a + p]
        xt = np.ascontiguousarray(
            shard.reshape(SLABS, N, 32, 128).transpose(0, 3, 2, 1)
        ).astype(np_in).reshape(SLABS, 128, 32 * N)
        in_maps.append({"xt": xt, "B1": B1, "B2": B2})
    return in_maps


def _unscramble(results, cfg=None):
    cfg = dict(CFG, **(cfg or {}))
    OB = cfg["out_batch"]
    NCB = 32 // OB
    outs = []
    for i in range(N_CORES):
        Y = np.asarray(results[i]["Y"])       # (SLABS, NCB, 128, OB*N)
        # Y[s, cb, 2*hi'+nu, j*N+n] = y[s*N+n, 64*hi' + 2*(cb*OB+j) + nu]
        y = (Y.reshape(SLABS, NCB, 64, 2, OB, N)
              .transpose(0, 5, 2, 1, 4, 3)
              .reshape(R, DIM))
        outs.append(y.astype(np.float32))
    return np.concatenate(outs, axis=0).reshape(4, 4096, DIM)


def kernel(x, H, _trace=False, _loop=0, _cfg=None):
    nc = _build_bass(MODE, loop=_loop, cfg=_cfg)
    in_maps = _prep_inputs(x, H, MODE, cfg=_cfg)
    res = run_bass_kernel_spmd(nc, in_maps, core_ids=list(range(N_CORES)),
                               trace=_trace)
    out = _unscramble(res.results, cfg=_cfg)
    if _trace:
        return out, res
    return out
